# revision 9
# baseline (speedup 1.0000x reference)
"""DeformableConv1D Trainium2 kernel (v3).

Math: the reference reduces to
    offset = conv1d(x, Wconv) + bconv
    m = mean(offset);  scale_k = relu(1 - |m + R_k|);  s = sum_k Wdef[k]*scale_k
    out = conv1d(s*x, Wconv) + bconv = s * conv_nobias(x) + bconv

Pipeline (per core, data-parallel over batch, 2 batches/core):

  Warm-up: a dummy 4B AllReduce fires at t=0 so the CC engine's library
  load + mesh setup + core-start skew are absorbed during the load phase.

  Phase 1 (streaming x once): for each 2MB load tile (natural layout,
  4 row-groups per partition) run 32 PE transposes into PSUM; each
  [128,1024] psum group drains as a STRAIGHT contiguous copy (cast to
  fp16) into the resident tensor xt, accumulating per-(s',c) channel
  sums. xt's column order is therefore group-permuted: within each
  1024-block group, column j'*128 + P holds block P*8 + j'.

  Sum tail: reduce the accumulators, dot with the host-provided qc
  vector on the PE, AllReduce 4B across the 8 cores (warm by now),
  then compute the scalar s on device.

  Phase 2 (polyphase conv, group-tiled): for group grp, tile j': PSUM
  partition P holds block grp*1024 + P*8 + j'. A-weights matmul on
  xt[grp, j'] (contiguous 128 cols), B-weights on xt[grp, j'+1]; the
  j'=7 tile takes B from xt[grp, j'=0] shifted one column plus a 1-col
  edge matmul reading the next group's first block. PSUM drains
  unscaled into a resident fp16 out buffer, so compute never throttles
  on stores. Groups are emitted with one tile of lag so the PE
  interleaves transposes and conv matmuls.

  Store: after the AllReduce, scale the fp16 out buffer in place by s
  (chunks alternate DVE/ACT) and DMA out as fp16 with 2KB contiguous
  per-partition lines (partition P of group grp holds 32 consecutive
  output rows). The host casts back to fp32.

bconv is all-zero in this problem; if not, it is added on the host.
"""

import numpy as np

import concourse.bacc as bacc
import concourse.bass as bass
import concourse.mybir as mybir
import concourse.tile as tile
from concourse.bass_utils import run_bass_kernel_spmd

FP = mybir.dt.float32
CONV_DT = mybir.dt.float16

N_CORES = 8
B_TOTAL = 16
T = 65536
C = 32
F = 32
K = 5

BPC = B_TOTAL // N_CORES      # batches per core
U = 32                        # sub-tiles ([128,128] fp32) per load tile
NT = T // (4 * 128 * U)       # load tiles per batch (4)
QB = T // 4                   # blocks per batch (16384)
NTILES = BPC * NT             # total load tiles (8)
NGRP = QB // 1024             # block groups per batch (16)


def build_kernel():
    nc = bacc.Bacc(
        "TRN2",
        target_bir_lowering=False,
        debug=False,
        enable_asserts=False,
        num_devices=N_CORES,
    )
    x = nc.dram_tensor("x", [BPC, T, C], FP, kind="ExternalInput").ap()
    wa = nc.dram_tensor("wa", [128, 128], CONV_DT, kind="ExternalInput").ap()
    wb = nc.dram_tensor("wb", [128, 128], CONV_DT, kind="ExternalInput").ap()
    ident = nc.dram_tensor("ident", [128, 128], FP, kind="ExternalInput").ap()
    qcv = nc.dram_tensor("qcv", [128, 1], FP, kind="ExternalInput").ap()
    c1 = nc.dram_tensor("c1", [1, 1], FP, kind="ExternalInput").ap()
    taps = nc.dram_tensor("taps", [1, K], FP, kind="ExternalInput").ap()
    wdef = nc.dram_tensor("wdef", [1, K], FP, kind="ExternalInput").ap()
    # out[g, grp, P, j'*128+so*32+f] = row grp*4096 + P*32 + 4j' + so
    out = nc.dram_tensor("out", [BPC, NGRP, 128, 1024], CONV_DT,
                         kind="ExternalOutput").ap()
    # the scalar s, so the host can patch the 4 rows per group whose
    # B-tap (partition 127, j'=7) the device skips
    s_out = nc.dram_tensor("s_out", [1, 1], FP, kind="ExternalOutput").ap()

    # load tiles: batch g, tile tt, partition p holds 4 units of 32 rows
    x_v = x.rearrange("g (tt v p r) c -> g tt p v (r c)", v=4, p=128, r=32)

    with tile.TileContext(nc) as tc:
        with (
            tc.tile_pool(name="res", bufs=1) as res_pool,
            tc.tile_pool(name="xload", bufs=3) as xload_pool,
            tc.tile_pool(name="consts", bufs=1) as cpool,
            tc.tile_pool(name="pst", bufs=2, space="PSUM") as pst_pool,
            tc.tile_pool(name="ps", bufs=3, space="PSUM") as ps_pool,
            tc.tile_pool(name="psmisc", bufs=1, space="PSUM") as psmisc_pool,
            tc.tile_pool(name="dram", bufs=1, space="DRAM") as dram_pool,
        ):
            # ---- warm-up collective: per-core singleton groups, so the CC
            # library loads at t=0 without any cross-core mesh and without
            # blocking the real AllReduce behind a slow warmup mesh ----
            warm_in = dram_pool.tile([1, 1], FP)
            warm_out = dram_pool.tile([1, 1], FP, addr_space="Shared")
            nc.gpsimd.collective_compute(
                "AllReduce",
                mybir.AluOpType.add,
                replica_groups=[[i] for i in range(N_CORES)],
                ins=[warm_in.opt()],
                outs=[warm_out.opt()],
            )

            # resident transposed x, fp16, group-permuted column order
            xt = res_pool.tile([128, BPC * QB], CONV_DT)
            # resident unscaled conv output, fp16, same column order
            outbuf = res_pool.tile([128, BPC * QB], CONV_DT)

            # first two loads before consts so the DMA rings start instantly
            xtiles = []
            for ti in range(2):
                g, tt = divmod(ti, NT)
                xt_l = xload_pool.tile([128, U * 128], FP, name="xl")
                nc.sync.dma_start(
                    xt_l.rearrange("p (v rc) -> p v rc", v=4), x_v[g, tt]
                )
                xtiles.append(xt_l)

            identity = cpool.tile([128, 128], FP)
            nc.gpsimd.dma_start(identity[:], ident[:])
            wa_t = cpool.tile([128, 128], CONV_DT)
            nc.gpsimd.dma_start(wa_t[:], wa[:])
            wb_t = cpool.tile([128, 128], CONV_DT)
            nc.gpsimd.dma_start(wb_t[:], wb[:])
            qcv_t = cpool.tile([128, 1], FP)
            nc.gpsimd.dma_start(qcv_t[:], qcv[:])
            c1_t = cpool.tile([1, 1], FP)
            nc.gpsimd.dma_start(c1_t[:], c1[:])
            taps_t = cpool.tile([1, K], FP)
            nc.gpsimd.dma_start(taps_t[:], taps[:])
            wdef_t = cpool.tile([1, K], FP)
            nc.gpsimd.dma_start(wdef_t[:], wdef[:])

            acc = cpool.tile([128, NTILES * 4], FP)
            nc.vector.memset(acc[:], 0.0)
            psmisc = psmisc_pool.tile([128, 1], FP)

            nco = 0

            def emit_mm_group(g, grp):
                """Conv matmuls + drains for one 1024-block group."""
                nonlocal nco
                gbase = g * QB + grp * 1024
                for half in range(2):
                    po = ps_pool.tile([128, 512], FP, name="ps")
                    for i in range(4):
                        jp = half * 4 + i
                        colA = gbase + jp * 128
                        sl = po[:, i * 128 : (i + 1) * 128]
                        nc.tensor.matmul(
                            sl, xt[:, colA : colA + 128], wa_t[:],
                            start=True, stop=False,
                        )
                        if jp < 7:
                            nc.tensor.matmul(
                                sl, xt[:, colA + 128 : colA + 256], wb_t[:],
                                start=False, stop=True,
                            )
                        else:
                            # B taps of block P*8+7 are blocks {(P+1)*8};
                            # partition 127 (next group's first block) is
                            # left A-only and patched on the host
                            nc.tensor.matmul(
                                po[0:127, i * 128 : (i + 1) * 128],
                                xt[:, gbase + 1 : gbase + 128], wb_t[:],
                                start=False, stop=True,
                            )
                    dst = outbuf[:, gbase + half * 512 : gbase + (half + 1) * 512]
                    if nco % 2 == 0:
                        nc.vector.tensor_copy(dst, po[:])
                    else:
                        nc.scalar.activation(
                            dst, po[:], mybir.ActivationFunctionType.Copy
                        )
                    nco += 1

            # ---- Phase 1 + interleaved conv groups (one tile of lag) ----
            ncopy = 0
            for ti in range(NTILES):
                g, tt = divmod(ti, NT)
                if ti < 2:
                    xtile = xtiles[ti]
                else:
                    xtile = xload_pool.tile([128, U * 128], FP, name="xl")
                    nc.sync.dma_start(
                        xtile.rearrange("p (v rc) -> p v rc", v=4), x_v[g, tt]
                    )
                for v in range(4):
                    pt = pst_pool.tile([128, 1024], FP, name="pt")
                    for j in range(8):
                        nc.tensor.transpose(
                            pt[:, j * 128 : (j + 1) * 128],
                            xtile[:, v * 1024 + j * 128 : v * 1024 + (j + 1) * 128],
                            identity[:],
                        )
                    base = g * QB + (tt * 4 + v) * 1024
                    if v == 3:
                        nc.vector.tensor_reduce(
                            acc[:, ncopy : ncopy + 1],
                            pt[:],
                            axis=mybir.AxisListType.X,
                            op=mybir.AluOpType.add,
                        )
                        nc.vector.tensor_copy(xt[:, base : base + 1024], pt[:])
                    else:
                        nc.scalar.activation(
                            xt[:, base : base + 1024],
                            pt[:],
                            mybir.ActivationFunctionType.Copy,
                            accum_out=acc[:, ncopy : ncopy + 1],
                        )
                    ncopy += 1
                if ti == NTILES - 1:
                    # sum tail -> real AllReduce (warm by now); emitted
                    # before this tile's conv groups so the trigger rides
                    # directly on the last drains
                    localsum = cpool.tile([128, 1], FP)
                    nc.vector.tensor_reduce(
                        localsum[:],
                        acc[:],
                        axis=mybir.AxisListType.X,
                        op=mybir.AluOpType.add,
                    )
                    nc.tensor.matmul(psmisc[0:1, 0:1], localsum[:], qcv_t[:])
                    mloc = cpool.tile([1, 1], FP)
                    nc.vector.tensor_copy(mloc[:], psmisc[0:1, 0:1])
                    ar_in = dram_pool.tile([1, 1], FP)
                    ar_out = dram_pool.tile([1, 1], FP, addr_space="Shared")
                    nc.gpsimd.dma_start(ar_in[:], mloc[:])
                    nc.gpsimd.collective_compute(
                        "AllReduce",
                        mybir.AluOpType.add,
                        replica_groups=[list(range(N_CORES))],
                        ins=[ar_in.opt()],
                        outs=[ar_out.opt()],
                    )
                    mg = cpool.tile([1, 1], FP)
                    nc.sync.dma_start(mg[:], ar_out[:])
                for grp in range(4 * tt, 4 * tt + 4):
                    emit_mm_group(g, grp)

            # ---- s = sum_k Wdef[k]*relu(1-|m+R_k|), broadcast to s_b ----
            s_b = cpool.tile([128, 1], FP)
            m1 = cpool.tile([1, 1], FP)
            nc.vector.tensor_tensor(m1[:], mg[:], c1_t[:], op=mybir.AluOpType.add)
            t1 = cpool.tile([1, K], FP)
            nc.vector.tensor_scalar_add(t1[:], taps_t[:], m1[:])
            t2 = cpool.tile([1, K], FP)
            nc.scalar.activation(t2[:], t1[:], mybir.ActivationFunctionType.Abs)
            t3 = cpool.tile([1, K], FP)
            nc.vector.tensor_scalar(
                t3[:], t2[:], -1.0, 1.0,
                op0=mybir.AluOpType.mult, op1=mybir.AluOpType.add,
            )
            t4 = cpool.tile([1, K], FP)
            nc.vector.tensor_scalar_max(t4[:], t3[:], 0.0)
            t5 = cpool.tile([1, K], FP)
            nc.vector.tensor_tensor(t5[:], t4[:], wdef_t[:], op=mybir.AluOpType.mult)
            s11 = cpool.tile([1, 1], FP)
            nc.vector.tensor_reduce(
                s11[:], t5[:], axis=mybir.AxisListType.X, op=mybir.AluOpType.add
            )
            nc.gpsimd.partition_broadcast(s_b[:], s11[:])
            nc.sync.dma_start(s_out[:], s11[:])

            # ---- scale in place + store (fp16, 2KB per-partition lines) ----
            for ch in range(BPC * NGRP):
                g, grp = divmod(ch, NGRP)
                c0 = g * QB + grp * 1024
                sl = outbuf[:, c0 : c0 + 1024]
                if ch % 2 == 0:
                    nc.vector.tensor_scalar_mul(sl, sl, s_b[:])
                else:
                    nc.scalar.activation(
                        sl, sl, mybir.ActivationFunctionType.Copy, scale=s_b[:]
                    )
                nc.sync.dma_start(out[g, grp], sl)

    nc.compile()
    return nc


_NC_CACHE = None
_LAST_IN_MAPS = None


def _get_nc():
    global _NC_CACHE
    if _NC_CACHE is None:
        _NC_CACHE = build_kernel()
    return _NC_CACHE


def _host_consts(x, Wconv, bconv):
    Tout = T - K + 1
    Ntot = B_TOTAL * Tout * F
    Wsum = Wconv.sum(axis=2).astype(np.float64)  # (K, C)
    head = x[:, : K - 1, :].astype(np.float64).sum(axis=0)  # (4, C)
    tail = x[:, T - (K - 1) :, :].astype(np.float64).sum(axis=0)  # (4, C)
    pre = np.concatenate([np.zeros((1, C)), np.cumsum(head, axis=0)], axis=0)
    suf = np.concatenate([np.zeros((1, C)), np.cumsum(tail[::-1], axis=0)], axis=0)
    edge = 0.0
    for k in range(K):
        edge += (Wsum[k] * (pre[k] + suf[K - 1 - k])).sum()
    qc = (Wsum.sum(axis=0) / Ntot).astype(np.float32)
    qcvec = np.tile(qc, 4).reshape(128, 1)
    c1 = np.float32(-edge / Ntot + float(np.mean(bconv)))
    return qcvec, np.array([[c1]], np.float32)


def _build_ab(Wconv):
    A = np.zeros((128, 128), np.float32)
    B = np.zeros((128, 128), np.float32)
    for sp in range(4):
        for so in range(4):
            k = sp - so
            if 0 <= k < K:
                A[sp * 32 : (sp + 1) * 32, so * 32 : (so + 1) * 32] = Wconv[k]
            k2 = sp - so + 4
            if 0 <= k2 < K:
                B[sp * 32 : (sp + 1) * 32, so * 32 : (so + 1) * 32] = Wconv[k2]
    return A.astype(np.float16), B.astype(np.float16)


def kernel(x, Wconv, bconv, Wdef):
    x = np.ascontiguousarray(np.asarray(x, np.float32))
    Wconv = np.asarray(Wconv, np.float32)
    bconv = np.asarray(bconv, np.float32)
    Wdef = np.asarray(Wdef, np.float32)

    nc = _get_nc()
    A, B = _build_ab(Wconv)
    qcvec, c1 = _host_consts(x, Wconv, bconv)
    ident = np.eye(128, dtype=np.float32)
    taps = (np.arange(K, dtype=np.float32) - (K // 2)).reshape(1, K)
    wdef_r = Wdef.reshape(1, K).astype(np.float32)

    in_maps = []
    for core in range(N_CORES):
        in_maps.append(
            {
                "x": x[core * BPC : (core + 1) * BPC],
                "wa": A,
                "wb": B,
                "ident": ident,
                "qcv": qcvec,
                "c1": c1,
                "taps": taps,
                "wdef": wdef_r,
            }
        )
    global _LAST_IN_MAPS
    _LAST_IN_MAPS = in_maps
    res = run_bass_kernel_spmd(nc, in_maps, list(range(N_CORES)))
    Tout = T - K + 1
    out = np.empty((B_TOTAL, Tout, F), np.float32)
    for core in range(N_CORES):
        o = res.results[core]["out"]  # (BPC, 16, 128, 1024) fp16
        # flat (grp, P, j', so, f) lexicographic IS row-major order
        o = o.reshape(BPC, T, F)
        out[core * BPC : (core + 1) * BPC] = o[:, :Tout, :].astype(np.float32)

    # patch the rows whose cross-group B-tap the device skips: blocks
    # q = grp*1024 + 1023 for grp < NGRP-1, rows 4q..4q+3
    s_val = np.float32(res.results[0]["s_out"][0, 0])
    rws = np.array(
        [grp * 4096 + 4092 + so for grp in range(NGRP - 1) for so in range(4)]
    )
    win = np.stack([x[:, r : r + K, :] for r in rws])  # (R, B, K, C)
    patched = np.einsum("rbkc,kcf->brf", win, Wconv) * s_val
    out[:, rws, :] = patched

    if np.any(bconv):
        out += bconv.reshape(1, 1, F)
    return out


# revision 16
# speedup vs baseline: 1.1092x; 1.1092x over previous
"""DeformableConv1D Trainium2 kernel (v3).

Math: the reference reduces to
    offset = conv1d(x, Wconv) + bconv
    m = mean(offset);  scale_k = relu(1 - |m + R_k|);  s = sum_k Wdef[k]*scale_k
    out = conv1d(s*x, Wconv) + bconv = s * conv_nobias(x) + bconv

Pipeline (per core, data-parallel over batch, 2 batches/core):

  Warm-up: a dummy 4B AllReduce fires at t=0 so the CC engine's library
  load + mesh setup + core-start skew are absorbed during the load phase.

  Phase 1 (streaming x once): for each 2MB load tile (natural layout,
  4 row-groups per partition) run 32 PE transposes into PSUM; each
  [128,1024] psum group drains as a STRAIGHT contiguous copy (cast to
  fp16) into the resident tensor xt, accumulating per-(s',c) channel
  sums. xt's column order is therefore group-permuted: within each
  1024-block group, column j'*128 + P holds block P*8 + j'.

  Sum tail: reduce the accumulators, dot with the host-provided qc
  vector on the PE, AllReduce 4B across the 8 cores (warm by now),
  then compute the scalar s on device.

  Phase 2 (polyphase conv, group-tiled): for group grp, tile j': PSUM
  partition P holds block grp*1024 + P*8 + j'. A-weights matmul on
  xt[grp, j'] (contiguous 128 cols), B-weights on xt[grp, j'+1]; the
  j'=7 tile takes B from xt[grp, j'=0] shifted one column plus a 1-col
  edge matmul reading the next group's first block. PSUM drains
  unscaled into a resident fp16 out buffer, so compute never throttles
  on stores. Groups are emitted with one tile of lag so the PE
  interleaves transposes and conv matmuls.

  Store: after the AllReduce, scale the fp16 out buffer in place by s
  (chunks alternate DVE/ACT) and DMA out as fp16 with 2KB contiguous
  per-partition lines (partition P of group grp holds 32 consecutive
  output rows). The host casts back to fp32.

bconv is all-zero in this problem; if not, it is added on the host.
"""

import numpy as np

import concourse.bacc as bacc
import concourse.bass as bass
import concourse.mybir as mybir
import concourse.tile as tile
from concourse.bass_utils import run_bass_kernel_spmd

FP = mybir.dt.float32
CONV_DT = mybir.dt.float16

N_CORES = 8
B_TOTAL = 16
T = 65536
C = 32
F = 32
K = 5

BPC = B_TOTAL // N_CORES      # batches per core
U = 32                        # sub-tiles ([128,128] fp32) per load tile
NT = T // (4 * 128 * U)       # load tiles per batch (4)
QB = T // 4                   # blocks per batch (16384)
NTILES = BPC * NT             # total load tiles (8)
NGRP = QB // 1024             # block groups per batch (16)


def build_kernel():
    nc = bacc.Bacc(
        "TRN2",
        target_bir_lowering=False,
        debug=False,
        enable_asserts=False,
        num_devices=N_CORES,
    )
    x = nc.dram_tensor("x", [BPC, T, C], CONV_DT, kind="ExternalInput").ap()
    wa = nc.dram_tensor("wa", [128, 128], CONV_DT, kind="ExternalInput").ap()
    wb = nc.dram_tensor("wb", [128, 128], CONV_DT, kind="ExternalInput").ap()
    ident = nc.dram_tensor("ident", [128, 128], CONV_DT, kind="ExternalInput").ap()
    qcv = nc.dram_tensor("qcv", [128, 1], FP, kind="ExternalInput").ap()
    c1 = nc.dram_tensor("c1", [1, 1], FP, kind="ExternalInput").ap()
    taps = nc.dram_tensor("taps", [1, K], FP, kind="ExternalInput").ap()
    wdef = nc.dram_tensor("wdef", [1, K], FP, kind="ExternalInput").ap()
    # out[g, grp, P, j'*128+so*32+f] = row grp*4096 + P*32 + 4j' + so
    out = nc.dram_tensor("out", [BPC, NGRP, 128, 1024], CONV_DT,
                         kind="ExternalOutput").ap()
    # the scalar s, so the host can patch the 4 rows per group whose
    # B-tap (partition 127, j'=7) the device skips
    s_out = nc.dram_tensor("s_out", [1, 1], FP, kind="ExternalOutput").ap()

    # load tiles: batch g, tile tt, partition p holds 4 units of 32 rows
    x_v = x.rearrange("g (tt v p r) c -> g tt p v (r c)", v=4, p=128, r=32)

    with tile.TileContext(nc) as tc:
        with (
            tc.tile_pool(name="res", bufs=1) as res_pool,
            tc.tile_pool(name="xload", bufs=3) as xload_pool,
            tc.tile_pool(name="consts", bufs=1) as cpool,
            tc.tile_pool(name="pst", bufs=2, space="PSUM") as pst_pool,
            tc.tile_pool(name="ps", bufs=3, space="PSUM") as ps_pool,
            tc.tile_pool(name="psmisc", bufs=1, space="PSUM") as psmisc_pool,
            tc.tile_pool(name="dram", bufs=1, space="DRAM") as dram_pool,
        ):
            # resident transposed x, fp16, group-permuted column order
            xt = res_pool.tile([128, BPC * QB], CONV_DT)
            # resident unscaled conv output, fp16, same column order
            outbuf = res_pool.tile([128, BPC * QB], CONV_DT)

            # first two loads before consts so the DMA rings start instantly
            xtiles = []
            for ti in range(2):
                g, tt = divmod(ti, NT)
                xt_l = xload_pool.tile([128, U * 128], CONV_DT, name="xl")
                nc.sync.dma_start(
                    xt_l.rearrange("p (v rc) -> p v rc", v=4), x_v[g, tt]
                )
                xtiles.append(xt_l)

            identity = cpool.tile([128, 128], CONV_DT)
            nc.gpsimd.dma_start(identity[:], ident[:])
            wa_t = cpool.tile([128, 128], CONV_DT)
            nc.gpsimd.dma_start(wa_t[:], wa[:])
            wb_t = cpool.tile([128, 128], CONV_DT)
            nc.gpsimd.dma_start(wb_t[:], wb[:])
            qcv_t = cpool.tile([128, 1], FP)
            nc.gpsimd.dma_start(qcv_t[:], qcv[:])
            c1_t = cpool.tile([1, 1], FP)
            nc.gpsimd.dma_start(c1_t[:], c1[:])
            taps_t = cpool.tile([1, K], FP)
            nc.gpsimd.dma_start(taps_t[:], taps[:])
            wdef_t = cpool.tile([1, K], FP)
            nc.gpsimd.dma_start(wdef_t[:], wdef[:])

            acc = cpool.tile([128, NTILES * 4], FP)
            nc.vector.memset(acc[:], 0.0)
            psmisc = psmisc_pool.tile([128, 1], FP)

            nco = 0

            def emit_mm_group(g, grp):
                """Conv matmuls + drains for one 1024-block group."""
                nonlocal nco
                gbase = g * QB + grp * 1024
                for half in range(2):
                    po = ps_pool.tile([128, 512], FP, name="ps")
                    for i in range(4):
                        jp = half * 4 + i
                        colA = gbase + jp * 128
                        sl = po[:, i * 128 : (i + 1) * 128]
                        nc.tensor.matmul(
                            sl, xt[:, colA : colA + 128], wa_t[:],
                            start=True, stop=False,
                        )
                        if jp < 7:
                            nc.tensor.matmul(
                                sl, xt[:, colA + 128 : colA + 256], wb_t[:],
                                start=False, stop=True,
                            )
                        else:
                            # B taps of block P*8+7 are blocks {(P+1)*8};
                            # partition 127 (next group's first block) is
                            # left A-only and patched on the host
                            nc.tensor.matmul(
                                po[0:127, i * 128 : (i + 1) * 128],
                                xt[:, gbase + 1 : gbase + 128], wb_t[:],
                                start=False, stop=True,
                            )
                    dst = outbuf[:, gbase + half * 512 : gbase + (half + 1) * 512]
                    if nco % 2 == 0:
                        nc.vector.tensor_copy(dst, po[:])
                    else:
                        nc.scalar.activation(
                            dst, po[:], mybir.ActivationFunctionType.Copy
                        )
                    nco += 1

            # ---- Phase 1 + interleaved conv groups (one tile of lag) ----
            ncopy = 0
            for ti in range(NTILES):
                g, tt = divmod(ti, NT)
                if ti < 2:
                    xtile = xtiles[ti]
                else:
                    xtile = xload_pool.tile([128, U * 128], CONV_DT, name="xl")
                    nc.sync.dma_start(
                        xtile.rearrange("p (v rc) -> p v rc", v=4), x_v[g, tt]
                    )
                for v in range(4):
                    pt = pst_pool.tile([128, 1024], CONV_DT, name="pt")
                    for j in range(8):
                        nc.tensor.transpose(
                            pt[:, j * 128 : (j + 1) * 128],
                            xtile[:, v * 1024 + j * 128 : v * 1024 + (j + 1) * 128],
                            identity[:],
                        )
                    base = g * QB + (tt * 4 + v) * 1024
                    if v == 3:
                        nc.vector.tensor_reduce(
                            acc[:, ncopy : ncopy + 1],
                            pt[:],
                            axis=mybir.AxisListType.X,
                            op=mybir.AluOpType.add,
                        )
                        nc.vector.tensor_copy(xt[:, base : base + 1024], pt[:])
                    else:
                        nc.scalar.activation(
                            xt[:, base : base + 1024],
                            pt[:],
                            mybir.ActivationFunctionType.Copy,
                            accum_out=acc[:, ncopy : ncopy + 1],
                        )
                    ncopy += 1
                if ti == NTILES - 1:
                    # sum tail -> real AllReduce (warm by now); emitted
                    # before this tile's conv groups so the trigger rides
                    # directly on the last drains
                    localsum = cpool.tile([128, 1], FP)
                    nc.vector.tensor_reduce(
                        localsum[:],
                        acc[:],
                        axis=mybir.AxisListType.X,
                        op=mybir.AluOpType.add,
                    )
                    nc.tensor.matmul(psmisc[0:1, 0:1], localsum[:], qcv_t[:])
                    mloc = cpool.tile([1, 1], FP)
                    nc.vector.tensor_copy(mloc[:], psmisc[0:1, 0:1])
                    ar_in = dram_pool.tile([1, 1], FP)
                    ar_out = dram_pool.tile([1, 1], FP, addr_space="Shared")
                    nc.gpsimd.dma_start(ar_in[:], mloc[:])
                    nc.gpsimd.collective_compute(
                        "AllReduce",
                        mybir.AluOpType.add,
                        replica_groups=[list(range(N_CORES))],
                        ins=[ar_in.opt()],
                        outs=[ar_out.opt()],
                    )
                    mg = cpool.tile([1, 1], FP)
                    nc.sync.dma_start(mg[:], ar_out[:])
                for grp in range(4 * tt, 4 * tt + 4):
                    emit_mm_group(g, grp)

            # ---- s = sum_k Wdef[k]*relu(1-|m+R_k|), broadcast to s_b ----
            s_b = cpool.tile([128, 1], FP)
            m1 = cpool.tile([1, 1], FP)
            nc.vector.tensor_tensor(m1[:], mg[:], c1_t[:], op=mybir.AluOpType.add)
            t1 = cpool.tile([1, K], FP)
            nc.vector.tensor_scalar_add(t1[:], taps_t[:], m1[:])
            t2 = cpool.tile([1, K], FP)
            nc.scalar.activation(t2[:], t1[:], mybir.ActivationFunctionType.Abs)
            t3 = cpool.tile([1, K], FP)
            nc.vector.tensor_scalar(
                t3[:], t2[:], -1.0, 1.0,
                op0=mybir.AluOpType.mult, op1=mybir.AluOpType.add,
            )
            t4 = cpool.tile([1, K], FP)
            nc.vector.tensor_scalar_max(t4[:], t3[:], 0.0)
            t5 = cpool.tile([1, K], FP)
            nc.vector.tensor_tensor(t5[:], t4[:], wdef_t[:], op=mybir.AluOpType.mult)
            s11 = cpool.tile([1, 1], FP)
            nc.vector.tensor_reduce(
                s11[:], t5[:], axis=mybir.AxisListType.X, op=mybir.AluOpType.add
            )
            nc.gpsimd.partition_broadcast(s_b[:], s11[:])
            nc.sync.dma_start(s_out[:], s11[:])

            # ---- scale in place + store (fp16, 2KB per-partition lines) ----
            for ch in range(BPC * NGRP):
                g, grp = divmod(ch, NGRP)
                c0 = g * QB + grp * 1024
                sl = outbuf[:, c0 : c0 + 1024]
                if ch % 2 == 0:
                    nc.vector.tensor_scalar_mul(sl, sl, s_b[:])
                else:
                    nc.scalar.activation(
                        sl, sl, mybir.ActivationFunctionType.Copy, scale=s_b[:]
                    )
                nc.sync.dma_start(out[g, grp], sl)

    nc.compile()
    return nc


_NC_CACHE = None
_LAST_IN_MAPS = None


def _get_nc():
    global _NC_CACHE
    if _NC_CACHE is None:
        _NC_CACHE = build_kernel()
    return _NC_CACHE


def _host_consts(x, Wconv, bconv):
    Tout = T - K + 1
    Ntot = B_TOTAL * Tout * F
    Wsum = Wconv.sum(axis=2).astype(np.float64)  # (K, C)
    head = x[:, : K - 1, :].astype(np.float64).sum(axis=0)  # (4, C)
    tail = x[:, T - (K - 1) :, :].astype(np.float64).sum(axis=0)  # (4, C)
    pre = np.concatenate([np.zeros((1, C)), np.cumsum(head, axis=0)], axis=0)
    suf = np.concatenate([np.zeros((1, C)), np.cumsum(tail[::-1], axis=0)], axis=0)
    edge = 0.0
    for k in range(K):
        edge += (Wsum[k] * (pre[k] + suf[K - 1 - k])).sum()
    qc = (Wsum.sum(axis=0) / Ntot).astype(np.float32)
    qcvec = np.tile(qc, 4).reshape(128, 1)
    c1 = np.float32(-edge / Ntot + float(np.mean(bconv)))
    return qcvec, np.array([[c1]], np.float32)


def _build_ab(Wconv):
    A = np.zeros((128, 128), np.float32)
    B = np.zeros((128, 128), np.float32)
    for sp in range(4):
        for so in range(4):
            k = sp - so
            if 0 <= k < K:
                A[sp * 32 : (sp + 1) * 32, so * 32 : (so + 1) * 32] = Wconv[k]
            k2 = sp - so + 4
            if 0 <= k2 < K:
                B[sp * 32 : (sp + 1) * 32, so * 32 : (so + 1) * 32] = Wconv[k2]
    return A.astype(np.float16), B.astype(np.float16)


def kernel(x, Wconv, bconv, Wdef):
    x = np.ascontiguousarray(np.asarray(x, np.float32))
    Wconv = np.asarray(Wconv, np.float32)
    bconv = np.asarray(bconv, np.float32)
    Wdef = np.asarray(Wdef, np.float32)

    nc = _get_nc()
    A, B = _build_ab(Wconv)
    qcvec, c1 = _host_consts(x, Wconv, bconv)
    ident = np.eye(128, dtype=np.float16)
    taps = (np.arange(K, dtype=np.float32) - (K // 2)).reshape(1, K)
    wdef_r = Wdef.reshape(1, K).astype(np.float32)
    # the conv consumes x in fp16 on device anyway; uploading fp16 halves
    # the load traffic (only the fp32 channel-sum path sees ~1e-6 change)
    x16 = x.astype(np.float16)

    in_maps = []
    for core in range(N_CORES):
        in_maps.append(
            {
                "x": x16[core * BPC : (core + 1) * BPC],
                "wa": A,
                "wb": B,
                "ident": ident,
                "qcv": qcvec,
                "c1": c1,
                "taps": taps,
                "wdef": wdef_r,
            }
        )
    global _LAST_IN_MAPS
    _LAST_IN_MAPS = in_maps
    res = run_bass_kernel_spmd(nc, in_maps, list(range(N_CORES)))
    Tout = T - K + 1
    out = np.empty((B_TOTAL, Tout, F), np.float32)
    for core in range(N_CORES):
        o = res.results[core]["out"]  # (BPC, 16, 128, 1024) fp16
        # flat (grp, P, j', so, f) lexicographic IS row-major order
        o = o.reshape(BPC, T, F)
        out[core * BPC : (core + 1) * BPC] = o[:, :Tout, :].astype(np.float32)

    # patch the rows whose cross-group B-tap the device skips: blocks
    # q = grp*1024 + 1023 for grp < NGRP-1, rows 4q..4q+3
    s_val = np.float32(res.results[0]["s_out"][0, 0])
    rws = np.array(
        [grp * 4096 + 4092 + so for grp in range(NGRP - 1) for so in range(4)]
    )
    win = np.stack([x[:, r : r + K, :] for r in rws])  # (R, B, K, C)
    patched = np.einsum("rbkc,kcf->brf", win, Wconv) * s_val
    out[:, rws, :] = patched

    if np.any(bconv):
        out += bconv.reshape(1, 1, F)
    return out
